# revision 9
# baseline (speedup 1.0000x reference)
"""Trainium2 Bass kernel for ClusterRetrieval (retrieval_knn).

reference semantics (B=16384, N=16384, D=128, k=8):
    sims = Q @ C.T                      # [B, N] f32
    vals, idx = top_k(sims, 8)          # descending, ties -> lowest index
    ids = where(vals >= 0.0, idx, -1)   # int32
    return (ids, vals, sims)

Sharding: data-parallel over the 8 NeuronCores — Q split into 8 row shards
of 2048, centroids replicated. Per core: PE matmul (exact fp32, bit-matches
the reference) with D=128 on the contraction partitions; ACT copies PSUM ->
SBUF; sims stream to HBM per 128-row tile.

Top-8 per row avoids a second full DVE scan: one pool_max(16) scan gives
window maxima, max8+max_index over the 1024 window maxima find the <=8
windows hosting the top-8 (every hosting window's max IS a top-8 value, so
it ranks in the pooled top-8); those windows are gathered back from the
just-written sims rows in DRAM via indirect DMA, and exact max8/max_index
over the gathered 8x16 elements give values + absolute indices.
"""

import os
import sys

import numpy as np

for _p in ("/root/.axon_site", "/root/.axon_site/_ro/trn_rl_repo", "/opt/trn_rl_repo"):
    if os.path.isdir(_p) and _p not in sys.path:
        sys.path.append(_p)

B, N, D, K = 16384, 16384, 128, 8
N_CORES = 8
B_SH = B // N_CORES          # 2048 rows per core
P = 128                      # partitions
NT_Q = B_SH // P             # 16 query tiles per core
MM_N = 512                   # moving free dim per matmul
PS_W = 2048                  # psum tile width (4 banks)
N_PS = N // PS_W             # 8 psum passes per query tile
W = 32                       # top-k pool window
NW = N // W                  # 1024 windows per row

_CACHE = {}
last_run_info = {}


def _build_program():
    import concourse.mybir as mybir
    from concourse import bacc, bass
    from concourse.masks import make_identity
    from concourse.tile import TileContext

    f32 = mybir.dt.float32
    i32 = mybir.dt.int32
    u32 = mybir.dt.uint32
    Alu = mybir.AluOpType

    nc = bacc.Bacc("TRN2", target_bir_lowering=False)
    q = nc.dram_tensor("q", [B_SH, D], f32, kind="ExternalInput")
    c = nc.dram_tensor("c", [N, D], f32, kind="ExternalInput")
    # one sims tensor per query tile: keeps the indirect gather's declared
    # read range (whole tensor, offset 0) from WAR-serializing later tiles'
    # sims writes against this tile's gather.
    sims = [
        nc.dram_tensor(f"sims{ti}", [P, N], f32, kind="ExternalOutput")
        for ti in range(NT_Q)
    ]
    vals = nc.dram_tensor("vals", [B_SH, K], f32, kind="ExternalOutput")
    ids = nc.dram_tensor("ids", [B_SH, K], i32, kind="ExternalOutput")

    with TileContext(nc) as tc:
        with (
            tc.tile_pool(name="const", bufs=1) as const_pool,
            tc.tile_pool(name="ct", bufs=1) as ct_pool,
            tc.tile_pool(name="qt", bufs=1) as qt_pool,
        ):
            ident = const_pool.tile([P, P], f32)
            make_identity(nc, ident)
            neg1 = const_pool.tile([P, K], i32)
            nc.vector.memset(neg1, -1)
            # rowiota[p, s] = p * NW  (window-row base per partition)
            rowiota = const_pool.tile([P, K], u32)
            nc.gpsimd.iota(rowiota, pattern=[[0, K]], base=0,
                           channel_multiplier=NW)

            ct = ct_pool.tile([P, N], f32)     # C^T  (d on partitions)
            qt = qt_pool.tile([P, B_SH], f32)  # Q^T  (d on partitions)

            # ---- prologue: load + transpose C and Q into SBUF ----
            with (
                tc.tile_pool(name="ld", bufs=4) as ld_pool,
                tc.tile_pool(name="tp", bufs=4, space="PSUM") as tp_pool,
            ):
                for i in range(N // 512):
                    lt = ld_pool.tile([P, 512], f32, tag="ld")
                    src = c[i * 512:(i + 1) * 512, :].rearrange(
                        "(g p) d -> p g d", p=P
                    )
                    nc.sync.dma_start(lt.rearrange("p (g d) -> p g d", g=4), src)
                    for g in range(4):
                        pt = tp_pool.tile([P, P], f32)
                        nc.tensor.transpose(pt, lt[:, g * P:(g + 1) * P], ident)
                        nc.any.tensor_copy(
                            ct[:, (i * 4 + g) * P:(i * 4 + g + 1) * P], pt
                        )
                for i in range(B_SH // 512):
                    lt = ld_pool.tile([P, 512], f32, tag="ld")
                    src = q[i * 512:(i + 1) * 512, :].rearrange(
                        "(g p) d -> p g d", p=P
                    )
                    nc.sync.dma_start(lt.rearrange("p (g d) -> p g d", g=4), src)
                    for g in range(4):
                        pt = tp_pool.tile([P, P], f32)
                        nc.tensor.transpose(pt, lt[:, g * P:(g + 1) * P], ident)
                        nc.any.tensor_copy(
                            qt[:, (i * 4 + g) * P:(i * 4 + g + 1) * P], pt
                        )

            # ---- main loop ----
            with (
                tc.tile_pool(name="sim", bufs=2) as sim_pool,
                tc.tile_pool(name="ps", bufs=2, space="PSUM") as ps_pool,
                tc.tile_pool(name="pmx", bufs=2) as pm_pool,
                tc.tile_pool(name="tk", bufs=2) as tk_pool,
            ):
                for ti in range(NT_Q):
                    r0 = ti * P
                    st = sim_pool.tile([P, N], f32, tag="st")
                    lhsT = qt[:, r0:r0 + P]
                    for pi in range(N_PS):
                        pt = ps_pool.tile([P, PS_W], f32, tag="ps")
                        for bi in range(PS_W // MM_N):
                            n0 = bi * MM_N
                            nc.tensor.matmul(
                                pt[:, n0:n0 + MM_N],
                                lhsT=lhsT,
                                rhs=ct[:, pi * PS_W + n0:pi * PS_W + n0 + MM_N],
                                start=True,
                                stop=True,
                            )
                        nc.any.tensor_copy(
                            st[:, pi * PS_W:(pi + 1) * PS_W], pt
                        )
                        nc.sync.dma_start(
                            sims[ti][:, pi * PS_W:(pi + 1) * PS_W],
                            st[:, pi * PS_W:(pi + 1) * PS_W],
                        )

                    # window maxima; <=8 windows host the row's top-8 and
                    # each such window's max ranks in pm's top-8
                    pm = pm_pool.tile([P, NW], f32, tag="pm")
                    nc.vector.reduce_max(
                        out=pm,
                        in_=st.rearrange("p (w i) -> p w i", i=W),
                        axis=mybir.AxisListType.X,
                    )
                    pv = tk_pool.tile([P, K], f32, tag="pv")
                    nc.vector.max(pv, pm)
                    wid = tk_pool.tile([P, K], u32, tag="wid")
                    nc.vector.max_index(wid, pv, pm)

                    # gather the 8 candidate windows from this tile's sims
                    # rows in DRAM (window-row index = p*NW + wid)
                    offs = tk_pool.tile([P, K], u32, tag="offs")
                    nc.gpsimd.tensor_tensor(
                        out=offs, in0=wid, in1=rowiota, op=Alu.add
                    )
                    gt = tk_pool.tile([P, K, W], f32, tag="gt")
                    sims_view = sims[ti][:, :].rearrange(
                        "r (w i) -> (r w) i", i=W
                    )
                    for s in range(K):
                        nc.gpsimd.indirect_dma_start(
                            out=gt[:, s, :],
                            out_offset=None,
                            in_=sims_view,
                            in_offset=bass.IndirectOffsetOnAxis(
                                ap=offs[:, s:s + 1], axis=0
                            ),
                        )
                    gtf = gt.rearrange("p s i -> p (s i)")

                    # exact top-8 values + positions within the gathered 128
                    v8 = tk_pool.tile([P, K], f32, tag="v8")
                    nc.vector.max(v8, gtf)
                    pos = tk_pool.tile([P, K], u32, tag="pos")
                    nc.vector.max_index(pos, v8, gtf)

                    # abs index = wid[slot]*W + intra
                    slot = tk_pool.tile([P, K], u32, tag="slot")
                    nc.vector.tensor_scalar(
                        slot, pos, 5, None, op0=Alu.logical_shift_right
                    )
                    intra = tk_pool.tile([P, K], u32, tag="intra")
                    nc.vector.tensor_scalar(
                        intra, pos, W - 1, None, op0=Alu.bitwise_and
                    )
                    wbase = tk_pool.tile([P, K], u32, tag="wbase")
                    nc.vector.tensor_copy(
                        wbase, wid[:, 0:1].to_broadcast([P, K])
                    )
                    ms = tk_pool.tile([P, K], u32, tag="ms")
                    for s in range(1, K):
                        nc.vector.tensor_scalar(
                            ms, slot, s, None, op0=Alu.is_equal
                        )
                        nc.vector.copy_predicated(
                            wbase, ms, wid[:, s:s + 1].to_broadcast([P, K])
                        )
                    absu = tk_pool.tile([P, K], u32, tag="absu")
                    nc.vector.tensor_scalar(absu, wbase, W, None, op0=Alu.mult)
                    nc.vector.tensor_tensor(
                        out=absu, in0=absu, in1=intra, op=Alu.add
                    )

                    # ids = where(v8 >= 0, absu, -1), int32
                    di = tk_pool.tile([P, K], i32, tag="di")
                    nc.vector.tensor_copy(di, absu)
                    mneg = tk_pool.tile([P, K], u32, tag="mneg")
                    nc.vector.tensor_scalar(
                        mneg, v8, 0.0, None, op0=Alu.is_lt
                    )
                    nc.vector.copy_predicated(di, mneg, neg1)

                    nc.sync.dma_start(vals[r0:r0 + P, :], v8)
                    nc.sync.dma_start(ids[r0:r0 + P, :], di)

    nc.compile()
    return nc


def _get_program():
    if "nc" not in _CACHE:
        _CACHE["nc"] = _build_program()
    return _CACHE["nc"]


def run_sharded(q_full, c_full, trace=False, **spmd_kwargs):
    from concourse.bass_utils import run_bass_kernel_spmd

    nc = _get_program()
    q_full = np.ascontiguousarray(q_full, dtype=np.float32)
    c_full = np.ascontiguousarray(c_full, dtype=np.float32)
    in_maps = [
        {"q": q_full[i * B_SH:(i + 1) * B_SH], "c": c_full}
        for i in range(N_CORES)
    ]
    res = run_bass_kernel_spmd(
        nc, in_maps, core_ids=list(range(N_CORES)), trace=trace, **spmd_kwargs
    )
    last_run_info["exec_time_ns"] = res.exec_time_ns
    last_run_info["mean_exec_time_ns"] = res.mean_exec_time_ns
    last_run_info["trace"] = res.instructions_and_trace
    sims = np.concatenate(
        [
            np.concatenate([r[f"sims{ti}"] for ti in range(NT_Q)], axis=0)
            for r in res.results
        ],
        axis=0,
    )
    vals = np.concatenate([r["vals"] for r in res.results], axis=0)
    ids = np.concatenate([r["ids"] for r in res.results], axis=0)
    return ids, vals, sims


def kernel(query_embeddings, cluster_embeddings, top_k):
    k = min(int(top_k), N)
    assert k <= K, f"kernel compiled for top-{K}, got top_k={k}"
    ids, vals, sims = run_sharded(query_embeddings, cluster_embeddings)
    if k < K:
        ids, vals = ids[:, :k], vals[:, :k]
    return ids, vals, sims


# revision 10
# speedup vs baseline: 1.1327x; 1.1327x over previous
"""Trainium2 Bass kernel for ClusterRetrieval (retrieval_knn).

reference semantics (B=16384, N=16384, D=128, k=8):
    sims = Q @ C.T                      # [B, N] f32
    vals, idx = top_k(sims, 8)          # descending, ties -> lowest index
    ids = where(vals >= 0.0, idx, -1)   # int32
    return (ids, vals, sims)

Sharding: data-parallel over the 8 NeuronCores — Q split into 8 row shards
of 2048, centroids replicated. Per core: PE matmul (exact fp32, bit-matches
the reference) with D=128 on the contraction partitions; ACT copies PSUM ->
SBUF; sims stream to HBM per 128-row tile.

Top-8 per row avoids a second full DVE scan: one pool_max(16) scan gives
window maxima, max8+max_index over the 1024 window maxima find the <=8
windows hosting the top-8 (every hosting window's max IS a top-8 value, so
it ranks in the pooled top-8); those windows are gathered back from the
just-written sims rows in DRAM via indirect DMA, and exact max8/max_index
over the gathered 8x16 elements give values + absolute indices.
"""

import os
import sys

import numpy as np

for _p in ("/root/.axon_site", "/root/.axon_site/_ro/trn_rl_repo", "/opt/trn_rl_repo"):
    if os.path.isdir(_p) and _p not in sys.path:
        sys.path.append(_p)

B, N, D, K = 16384, 16384, 128, 8
N_CORES = 8
B_SH = B // N_CORES          # 2048 rows per core
P = 128                      # partitions
NT_Q = B_SH // P             # 16 query tiles per core
MM_N = 512                   # moving free dim per matmul
PS_W = 2048                  # psum tile width (4 banks)
N_PS = N // PS_W             # 8 psum passes per query tile
W = 32                       # top-k pool window
NW = N // W                  # 1024 windows per row

_CACHE = {}
last_run_info = {}


def _build_program():
    import concourse.mybir as mybir
    from concourse import bacc, bass
    from concourse.masks import make_identity
    from concourse.tile import TileContext

    f32 = mybir.dt.float32
    i32 = mybir.dt.int32
    u32 = mybir.dt.uint32
    Alu = mybir.AluOpType

    nc = bacc.Bacc("TRN2", target_bir_lowering=False)
    q = nc.dram_tensor("q", [B_SH, D], f32, kind="ExternalInput")
    c = nc.dram_tensor("c", [N, D], f32, kind="ExternalInput")
    # one sims tensor per query tile: keeps the indirect gather's declared
    # read range (whole tensor, offset 0) from WAR-serializing later tiles'
    # sims writes against this tile's gather.
    sims = [
        nc.dram_tensor(f"sims{ti}", [P, N], f32, kind="ExternalOutput")
        for ti in range(NT_Q)
    ]
    vals = nc.dram_tensor("vals", [B_SH, K], f32, kind="ExternalOutput")
    ids = nc.dram_tensor("ids", [B_SH, K], i32, kind="ExternalOutput")

    with TileContext(nc) as tc:
        with (
            tc.tile_pool(name="const", bufs=1) as const_pool,
            tc.tile_pool(name="ct", bufs=1) as ct_pool,
            tc.tile_pool(name="qt", bufs=1) as qt_pool,
        ):
            ident = const_pool.tile([P, P], f32)
            make_identity(nc, ident)
            neg1 = const_pool.tile([P, K], i32)
            nc.vector.memset(neg1, -1)
            # rowiota[p, s] = p * NW  (window-row base per partition)
            rowiota = const_pool.tile([P, K], u32)
            nc.gpsimd.iota(rowiota, pattern=[[0, K]], base=0,
                           channel_multiplier=NW)

            ct = ct_pool.tile([P, N], f32)     # C^T  (d on partitions)
            qt = qt_pool.tile([P, B_SH], f32)  # Q^T  (d on partitions)

            # ---- prologue: load + transpose C and Q into SBUF ----
            with (
                tc.tile_pool(name="ld", bufs=4) as ld_pool,
                tc.tile_pool(name="tp", bufs=4, space="PSUM") as tp_pool,
            ):
                for i in range(N // 512):
                    lt = ld_pool.tile([P, 512], f32, tag="ld")
                    src = c[i * 512:(i + 1) * 512, :].rearrange(
                        "(g p) d -> p g d", p=P
                    )
                    nc.sync.dma_start(lt.rearrange("p (g d) -> p g d", g=4), src)
                    for g in range(4):
                        pt = tp_pool.tile([P, P], f32)
                        nc.tensor.transpose(pt, lt[:, g * P:(g + 1) * P], ident)
                        nc.any.tensor_copy(
                            ct[:, (i * 4 + g) * P:(i * 4 + g + 1) * P], pt
                        )
                for i in range(B_SH // 512):
                    lt = ld_pool.tile([P, 512], f32, tag="ld")
                    src = q[i * 512:(i + 1) * 512, :].rearrange(
                        "(g p) d -> p g d", p=P
                    )
                    nc.sync.dma_start(lt.rearrange("p (g d) -> p g d", g=4), src)
                    for g in range(4):
                        pt = tp_pool.tile([P, P], f32)
                        nc.tensor.transpose(pt, lt[:, g * P:(g + 1) * P], ident)
                        nc.any.tensor_copy(
                            qt[:, (i * 4 + g) * P:(i * 4 + g + 1) * P], pt
                        )

            # ---- main loop ----
            with (
                tc.tile_pool(name="sim", bufs=2) as sim_pool,
                tc.tile_pool(name="ps", bufs=2, space="PSUM") as ps_pool,
                tc.tile_pool(name="pmx", bufs=2) as pm_pool,
                tc.tile_pool(name="tk", bufs=2) as tk_pool,
            ):
                WPP = PS_W // W  # windows per psum pass
                for ti in range(NT_Q):
                    r0 = ti * P
                    st = sim_pool.tile([P, N], f32, tag="st")
                    pm = pm_pool.tile([P, NW], f32, tag="pm")
                    lhsT = qt[:, r0:r0 + P]
                    for pi in range(N_PS):
                        pt = ps_pool.tile([P, PS_W], f32, tag="ps")
                        for bi in range(PS_W // MM_N):
                            n0 = bi * MM_N
                            nc.tensor.matmul(
                                pt[:, n0:n0 + MM_N],
                                lhsT=lhsT,
                                rhs=ct[:, pi * PS_W + n0:pi * PS_W + n0 + MM_N],
                                start=True,
                                stop=True,
                            )
                        sl = slice(pi * PS_W, (pi + 1) * PS_W)
                        nc.any.tensor_copy(st[:, sl], pt)
                        nc.sync.dma_start(sims[ti][:, sl], st[:, sl])
                        # incremental window maxima for this pass (overlaps
                        # later passes' matmuls instead of one post-hoc scan)
                        nc.vector.reduce_max(
                            out=pm[:, pi * WPP:(pi + 1) * WPP],
                            in_=st[:, sl].rearrange("p (w i) -> p w i", i=W),
                            axis=mybir.AxisListType.X,
                        )

                    # <=8 windows host the row's top-8 and each such
                    # window's max ranks in pm's top-8
                    pv = tk_pool.tile([P, K], f32, tag="pv")
                    nc.vector.max(pv, pm)
                    wid = tk_pool.tile([P, K], u32, tag="wid")
                    nc.vector.max_index(wid, pv, pm)

                    # gather the 8 candidate windows from this tile's sims
                    # rows in DRAM (window-row index = p*NW + wid)
                    offs = tk_pool.tile([P, K], u32, tag="offs")
                    nc.gpsimd.tensor_tensor(
                        out=offs, in0=wid, in1=rowiota, op=Alu.add
                    )
                    gt = tk_pool.tile([P, K, W], f32, tag="gt")
                    sims_view = sims[ti][:, :].rearrange(
                        "r (w i) -> (r w) i", i=W
                    )
                    for s in range(K):
                        nc.gpsimd.indirect_dma_start(
                            out=gt[:, s, :],
                            out_offset=None,
                            in_=sims_view,
                            in_offset=bass.IndirectOffsetOnAxis(
                                ap=offs[:, s:s + 1], axis=0
                            ),
                        )
                    gtf = gt.rearrange("p s i -> p (s i)")

                    # exact top-8 values + positions within the gathered 128
                    v8 = tk_pool.tile([P, K], f32, tag="v8")
                    nc.vector.max(v8, gtf)
                    pos = tk_pool.tile([P, K], u32, tag="pos")
                    nc.vector.max_index(pos, v8, gtf)

                    # abs index = wid[slot]*W + intra
                    slot = tk_pool.tile([P, K], u32, tag="slot")
                    nc.vector.tensor_scalar(
                        slot, pos, 5, None, op0=Alu.logical_shift_right
                    )
                    intra = tk_pool.tile([P, K], u32, tag="intra")
                    nc.vector.tensor_scalar(
                        intra, pos, W - 1, None, op0=Alu.bitwise_and
                    )
                    wbase = tk_pool.tile([P, K], u32, tag="wbase")
                    nc.vector.tensor_copy(
                        wbase, wid[:, 0:1].to_broadcast([P, K])
                    )
                    ms = tk_pool.tile([P, K], u32, tag="ms")
                    for s in range(1, K):
                        nc.vector.tensor_scalar(
                            ms, slot, s, None, op0=Alu.is_equal
                        )
                        nc.vector.copy_predicated(
                            wbase, ms, wid[:, s:s + 1].to_broadcast([P, K])
                        )
                    absu = tk_pool.tile([P, K], u32, tag="absu")
                    nc.vector.tensor_scalar(absu, wbase, W, None, op0=Alu.mult)
                    nc.vector.tensor_tensor(
                        out=absu, in0=absu, in1=intra, op=Alu.add
                    )

                    # ids = where(v8 >= 0, absu, -1), int32
                    di = tk_pool.tile([P, K], i32, tag="di")
                    nc.vector.tensor_copy(di, absu)
                    mneg = tk_pool.tile([P, K], u32, tag="mneg")
                    nc.vector.tensor_scalar(
                        mneg, v8, 0.0, None, op0=Alu.is_lt
                    )
                    nc.vector.copy_predicated(di, mneg, neg1)

                    nc.sync.dma_start(vals[r0:r0 + P, :], v8)
                    nc.sync.dma_start(ids[r0:r0 + P, :], di)

    nc.compile()
    return nc


def _get_program():
    if "nc" not in _CACHE:
        _CACHE["nc"] = _build_program()
    return _CACHE["nc"]


def run_sharded(q_full, c_full, trace=False, **spmd_kwargs):
    from concourse.bass_utils import run_bass_kernel_spmd

    nc = _get_program()
    q_full = np.ascontiguousarray(q_full, dtype=np.float32)
    c_full = np.ascontiguousarray(c_full, dtype=np.float32)
    in_maps = [
        {"q": q_full[i * B_SH:(i + 1) * B_SH], "c": c_full}
        for i in range(N_CORES)
    ]
    res = run_bass_kernel_spmd(
        nc, in_maps, core_ids=list(range(N_CORES)), trace=trace, **spmd_kwargs
    )
    last_run_info["exec_time_ns"] = res.exec_time_ns
    last_run_info["mean_exec_time_ns"] = res.mean_exec_time_ns
    last_run_info["trace"] = res.instructions_and_trace
    sims = np.concatenate(
        [
            np.concatenate([r[f"sims{ti}"] for ti in range(NT_Q)], axis=0)
            for r in res.results
        ],
        axis=0,
    )
    vals = np.concatenate([r["vals"] for r in res.results], axis=0)
    ids = np.concatenate([r["ids"] for r in res.results], axis=0)
    return ids, vals, sims


def kernel(query_embeddings, cluster_embeddings, top_k):
    k = min(int(top_k), N)
    assert k <= K, f"kernel compiled for top-{K}, got top_k={k}"
    ids, vals, sims = run_sharded(query_embeddings, cluster_embeddings)
    if k < K:
        ids, vals = ids[:, :k], vals[:, :k]
    return ids, vals, sims
